# revision 17
# baseline (speedup 1.0000x reference)
"""Trainium2 Bass kernel for nn_DCWTv2InferenceCache (segment-tree cached attention).

Sharding: tensor-parallel over the 16-head axis -> 8 cores x 2 heads.
The two big segment-tree nodes (98% of the value-cache bytes) stream from
HBM as fp8-e4m3 in a host-pre-arranged layout: SBUF partition = (token%64)
+ 64*(chunk-row parity), so ONE fixed fold matrix PP[p,k]=(k==p%64) serves
every block-sum matmul (walrus collapses the repeated LDWEIGHTS to a no-op)
and DoubleRow contracts two c-blocks per instruction (4 tokens/cycle).
Local window / small nodes / mid nodes stay f32, host-pre-arranged to
partition-major contiguous layouts so their DMAs are descriptor-light.
All input-derived scalars (depth-projected queries, softmax scales with
block-mean and 1/NT folded in) are host-computed and shipped in one consts
blob.  Output is head-sharded (2, 64) per core, gathered on host.  No
cross-device communication.
"""

import math
import os
import sys

if "/opt/trn_rl_repo" not in sys.path:
    sys.path.insert(0, "/opt/trn_rl_repo")

import numpy as np
import ml_dtypes

import concourse.bass as bass
import concourse.mybir as mybir
import concourse.tile as tile
from concourse import bacc
from concourse.bass_utils import run_bass_kernel_spmd

# --- problem constants (from the reference nn.Module) ---
MAX_LEN = 65536
NUM_HEADS = 16
HEAD_DIM = 64
K_MAX = 64
LOCAL_WINDOW = 512
LOG_N = 17
LEAF_START = 2**LOG_N

N_CORES = 8
HPC = NUM_HEADS // N_CORES        # heads per core = 2
F = HPC * HEAD_DIM                # feature width per core = 128
NTOK = 50000                      # v_tokens buffer length

CHUNK = 128                       # tokens per matmul tile (partition dim)
BLK = CHUNK * K_MAX               # 8192 tokens per c-block (stream path)

# stream-path arithmetic mode: "fp8dr" | "fp8" | "bf16"
MODE = os.environ.get("DCWT_MODE", "fp8dr")

f32 = mybir.dt.float32
AF = mybir.ActivationFunctionType
AX = mybir.AxisListType

_last_results = None  # stash for test harness introspection


def cover_set(pos):
    """O(log n) segment-tree nodes covering prefix [0..pos-1]: (start, L, depth),
    ascending start / descending L (binary decomposition of pos)."""
    if pos <= 0:
        return []
    l, r = LEAF_START, LEAF_START + min(pos, MAX_LEN)
    out = []
    while l < r:
        if l & 1:
            d = LOG_N - int(math.floor(math.log2(l)))
            out.append(((l << d) - LEAF_START, 1 << d, d))
            l += 1
        if r & 1:
            r -= 1
            d = LOG_N - int(math.floor(math.log2(r)))
            out.append(((r << d) - LEAF_START, 1 << d, d))
        l >>= 1
        r >>= 1
    return sorted(out)


def _split_nodes(pos):
    """stream nodes (L >= BLK, fp8 path) / old nodes (K_MAX < L < BLK, f32r
    PE path) / small nodes (L <= K_MAX, raw)."""
    nodes = cover_set(pos)
    stream = [(s, L, d) for (s, L, d) in nodes if L >= BLK]
    old = [(s, L, d) for (s, L, d) in nodes if K_MAX < L < BLK]
    small = [(s, L, d) for (s, L, d) in nodes if L <= K_MAX]
    return stream, old, small


def _cblob_layout(NT):
    """Column offsets inside the packed (128, W) f32 constants blob."""
    nt = max(NT, 1)
    off = {"ident": 0, "qbd": 128, "qdT": 130, "rs": 130 + 2 * nt}
    return off, 130 + 3 * nt


def _mode_dt():
    if MODE == "bf16":
        return mybir.dt.bfloat16, np.dtype(ml_dtypes.bfloat16)
    return mybir.dt.float8e4, np.dtype(ml_dtypes.float8_e4m3)


def _use_dr(L):
    return MODE == "fp8dr" and (L // BLK) >= 2


def _build_program(pos):
    """Build the single-core Bass/Tile program (same program for all 8 cores)."""
    stream, old, small = _split_nodes(pos)
    tree = stream + old + small
    NT = len(tree)
    n_loc = min(pos, LOCAL_WINDOW)
    assert n_loc % CHUNK == 0, "local window must be chunk-aligned for this build"
    NLC = n_loc // CHUNK
    inv_sqrt_d = 1.0 / math.sqrt(HEAD_DIM)
    st_dt, _ = _mode_dt()

    nc = bacc.Bacc("TRN2", target_bir_lowering=False, debug=False)

    # --- HBM tensors ---
    a_d = [
        nc.dram_tensor(f"a{i}", [CHUNK, (L // CHUNK) * F], st_dt,
                       kind="ExternalInput")
        for i, (s, L, _d) in enumerate(stream)
    ]
    pp8_d = nc.dram_tensor("pp8", [CHUNK, 2 * K_MAX], st_dt, kind="ExternalInput")
    OFF, CB_W = _cblob_layout(NT)
    cblob_d = nc.dram_tensor("cblob", [CHUNK, CB_W], f32, kind="ExternalInput")
    # f32 residual blob: local window chunks + (row-truncated) small nodes
    NS = len(small)
    vres_d = nc.dram_tensor("vres", [CHUNK, (NLC + max(NS, 1)) * F], f32,
                            kind="ExternalInput")
    # f32r blob: old-path fold matrix + old-node chunks, one DMA
    old_nch = [L_b // CHUNK for (_s, L_b, _d) in old]
    VO_W = K_MAX + sum(old_nch) * F
    vold_d = nc.dram_tensor("voldb", [CHUNK, VO_W], mybir.dt.float32r,
                            kind="ExternalInput")
    o = nc.dram_tensor("o", [HPC, HEAD_DIM], f32, kind="ExternalOutput")

    # host-baked python-float constants per tree node
    ms = [float(K_MAX) / L if L > K_MAX else 1.0 for (_s, L, _d) in tree]
    zfac = [float(NT) / m for m in ms]

    with tile.TileContext(nc) as tc:
        with (
            tc.tile_pool(name="consts", bufs=1) as cpool,
            tc.tile_pool(name="ep_sb", bufs=2) as spool,
            tc.tile_pool(name="xsb", bufs=3) as xpool,
            tc.tile_pool(name="acc_ps", bufs=1, space=bass.MemorySpace.PSUM) as apool,
            tc.tile_pool(name="ep_ps", bufs=1, space=bass.MemorySpace.PSUM) as eppool,
            tc.tile_pool(name="out_ps", bufs=1, space=bass.MemorySpace.PSUM) as opool,
        ):
            # ---- DMA issue: fp8 stream on the sync HWDGE ring, j-halves for
            # fine-grained PE chase; weights/consts/residuals on the scalar
            # ring (all host-pre-arranged partition-major => cheap descriptors).
            a_tiles = []
            for i, (s, L, _d) in enumerate(stream):
                nb = L // BLK
                if _use_dr(L):
                    at = cpool.tile([CHUNK, K_MAX, 2, (nb // 2) * F], st_dt,
                                    name=f"a{i}t", tag=f"a{i}t")
                    h1, h2 = at[:, 0 : K_MAX // 2, :, :], at[:, K_MAX // 2 :, :, :]
                else:
                    at = cpool.tile([CHUNK, K_MAX, nb * F], st_dt,
                                    name=f"a{i}t", tag=f"a{i}t")
                    h1, h2 = at[:, 0 : K_MAX // 2, :], at[:, K_MAX // 2 :, :]
                half = (K_MAX // 2) * nb * F
                nc.sync.dma_start(h1, a_d[i][:, 0:half])
                nc.sync.dma_start(h2, a_d[i][:, half:])
                a_tiles.append(at)

            pp8t = cpool.tile([CHUNK, 2, K_MAX], st_dt)
            nc.scalar.dma_start(pp8t[:], pp8_d[:])
            cb = cpool.tile([CHUNK, CB_W], f32)
            nc.scalar.dma_start(cb[:], cblob_d[:])
            ident_sb = cb[:, OFF["ident"] : OFF["ident"] + CHUNK]
            qbd_sb = cb[:, OFF["qbd"] : OFF["qbd"] + HPC]

            def qdT_slice(n):
                return cb[:, OFF["qdT"] + n * HPC : OFF["qdT"] + (n + 1) * HPC]

            rs_sb = cb[0:HPC, OFF["rs"] : OFF["rs"] + max(NT, 1)]

            vres = cpool.tile([CHUNK, NLC + max(NS, 1), F], f32)
            nc.scalar.dma_start(vres[:], vres_d[:])
            voldb = cpool.tile([CHUNK, VO_W], mybir.dt.float32r)
            nc.scalar.dma_start(voldb[:], vold_d[:])
            old_off = []
            off = K_MAX
            for nch in old_nch:
                old_off.append(off)
                off += nch * F

            # ---- cross-node output accumulator (2, 128) PSUM ----
            out_ps = opool.tile([HPC, F], f32)
            n_out_mm = NT + NLC
            out_mm = [0]  # running count, for start/stop flags

            def out_matmul(wT_sb_ap, f_sb_ap):
                nc.tensor.matmul(
                    out_ps[:], wT_sb_ap, f_sb_ap,
                    start=(out_mm[0] == 0), stop=(out_mm[0] == n_out_mm - 1),
                )
                out_mm[0] += 1

            def softmax_weights(s_ps_ap, K, node_i):
                """exp(scale*s) -> normalize; tree weights fold 1/NT and the
                block-mean factor via zfac (host-baked).  No max-subtraction:
                logits here are provably small."""
                ebd = xpool.tile([HPC, K], f32, tag="esb")
                zt = xpool.tile([HPC, 1], f32, tag="zt")
                if node_i >= 0:
                    nc.scalar.activation(
                        ebd[:], s_ps_ap, AF.Exp,
                        scale=rs_sb[:, node_i : node_i + 1], accum_out=zt[:],
                    )
                    zs = xpool.tile([HPC, 1], f32, tag="zs")
                    nc.scalar.mul(zs[:], zt[:], zfac[node_i])
                    zt = zs
                else:
                    nc.scalar.activation(
                        ebd[:], s_ps_ap, AF.Exp, scale=inv_sqrt_d,
                        accum_out=zt[:],
                    )
                rz = xpool.tile([HPC, 1], f32, tag="rz")
                nc.vector.reciprocal(rz[:], zt[:])
                w_sb = xpool.tile([HPC, K], f32, tag="wsb")
                nc.vector.tensor_scalar_mul(w_sb[:], ebd[:], rz[:])
                return w_sb

            def epi_A(node_i, f_sb_ap, K):
                """Transpose f, logits, softmax -> weights (PE ops: T1, M1)."""
                fT_ps = eppool.tile([F, K_MAX], f32, tag="fT_ps", bufs=1)
                nc.tensor.transpose(fT_ps[:, 0:K], f_sb_ap, ident_sb[0:K, 0:K])
                fT_sb = spool.tile([F, K_MAX], f32, tag="fT_sb")
                nc.scalar.copy(fT_sb[:, 0:K], fT_ps[:, 0:K])
                s_ps = eppool.tile([HPC, K_MAX], f32, tag="s_ps", bufs=1)
                nc.tensor.matmul(
                    s_ps[:, 0:K], qdT_slice(node_i), fT_sb[:, 0:K],
                    start=True, stop=True,
                )
                return softmax_weights(s_ps[:, 0:K], K, node_i)

            def epi_B(w_sb, f_sb_ap, K):
                """Weights transpose + output accumulation (PE ops: T2, M2)."""
                wT_ps = eppool.tile([K_MAX, HPC], f32, tag="wT_ps")
                nc.tensor.transpose(wT_ps[0:K, :], w_sb[:], ident_sb[0:HPC, 0:HPC])
                wT_sb = spool.tile([K_MAX, HPC], f32, tag="wT_sb")
                nc.scalar.copy(wT_sb[0:K, :], wT_ps[0:K, :])
                out_matmul(wT_sb[0:K, :], f_sb_ap)

            def tree_epilogue(node_i, f_sb_ap, K):
                epi_B(epi_A(node_i, f_sb_ap, K), f_sb_ap, K)

            # ================= emission schedule =================
            def emit_local():
                fTl_ps = eppool.tile([F, NLC * CHUNK], f32, tag="fT_ps", bufs=1)
                for c in range(NLC):
                    nc.tensor.transpose(
                        fTl_ps[:, c * CHUNK : (c + 1) * CHUNK], vres[:, c, :],
                        ident_sb[:],
                    )
                fTl_sb = spool.tile([F, NLC * CHUNK], f32, tag="fTl_sb")
                nc.scalar.copy(fTl_sb[:], fTl_ps[:])
                sl_ps = eppool.tile([HPC, NLC * CHUNK], f32, tag="s_ps", bufs=1)
                nc.tensor.matmul(sl_ps[:], qbd_sb, fTl_sb[:], start=True, stop=True)
                wl_sb = softmax_weights(sl_ps[:], n_loc, -1)
                for c in range(NLC):
                    wTl_ps = eppool.tile([CHUNK, HPC], f32, tag="wT_ps")
                    nc.tensor.transpose(
                        wTl_ps[:], wl_sb[:, c * CHUNK : (c + 1) * CHUNK],
                        ident_sb[0:HPC, 0:HPC],
                    )
                    wTl_sb = spool.tile([CHUNK, HPC], f32, tag="wTl_sb")
                    nc.scalar.copy(wTl_sb[:], wTl_ps[:])
                    out_matmul(wTl_sb[:], vres[:, c, :])

            def emit_old_node(node_i, oi, L):
                nch = L // CHUNK
                base = old_off[oi]
                ps2 = apool.tile([K_MAX, 2, F], f32, tag="acc")
                done, c = 0, 0
                while c < nch:
                    w = 2 if c + 2 <= nch else 1
                    nc.tensor.matmul(
                        ps2[:, 0:w, :], voldb[:, 0:K_MAX],
                        voldb[:, base + c * F : base + (c + w) * F],
                        start=(done == 0), stop=(done + w == nch),
                    )
                    done += w
                    c += w
                f_sb = spool.tile([K_MAX, F], f32, tag="fold")
                if nch > 1:
                    g = spool.tile([K_MAX, 2, F], f32, tag="gfold")
                    nc.scalar.copy(g[:], ps2[:])
                    nc.vector.tensor_add(f_sb[:], g[:, 0, :], g[:, 1, :])
                else:
                    nc.scalar.copy(f_sb[:], ps2[:, 0, :])
                tree_epilogue(node_i, f_sb[:], K_MAX)

            def emit_stream_mms(i, L, jlo, jhi):
                """Fold matmuls for stream node i, j in [jlo, jhi).  All MMs
                share the fixed PP weights -> LDWEIGHTS collapses to a no-op."""
                nb = L // BLK
                at = a_tiles[i]
                ncol = nb // 2 if _use_dr(L) else nb
                ps = apool.tile([K_MAX, ncol * F], f32, tag=f"st{i}")
                if _use_dr(L):
                    dr = mybir.MatmulPerfMode.DoubleRow
                    for j in range(jlo, jhi):
                        nc.tensor.matmul(
                            ps[:], pp8t[:], at[:, j, :, :],
                            start=(j == 0), stop=(j == K_MAX - 1), perf_mode=dr,
                        )
                else:
                    for j in range(jlo, jhi):
                        nc.tensor.matmul(
                            ps[:], pp8t[:, 0, :], at[:, j, :],
                            start=(j == 0), stop=(j == K_MAX - 1),
                        )
                return ps

            def stream_fold(i, ps, L):
                """Combine c-group partial sums -> (64, F) raw block-sum."""
                ncol = (L // BLK) // 2 if _use_dr(L) else L // BLK
                f_sb = spool.tile([K_MAX, F], f32, tag="fold")
                if ncol == 1:
                    nc.scalar.copy(f_sb[:], ps[:])
                    return f_sb
                g = spool.tile([K_MAX, ncol * F], f32, tag="gfold")
                nc.scalar.copy(g[:], ps[:])
                if ncol == 2:
                    nc.vector.tensor_add(f_sb[:], g[:, 0:F], g[:, F : 2 * F])
                else:  # ncol == 4
                    ha = spool.tile([K_MAX, F], f32, tag="ha")
                    nc.vector.tensor_add(ha[:], g[:, 0:F], g[:, F : 2 * F])
                    hb = spool.tile([K_MAX, F], f32, tag="hb")
                    nc.vector.tensor_add(hb[:], g[:, 2 * F : 3 * F],
                                         g[:, 3 * F : 4 * F])
                    nc.vector.tensor_add(f_sb[:], ha[:], hb[:])
                return f_sb

            emit_local()
            for si, (start_s, L_s, _d) in enumerate(small):
                tree_epilogue(len(stream) + len(old) + si,
                              vres[0:L_s, NLC + si, :], L_s)
            for oi, (start_b, L_b, _d) in enumerate(old):
                emit_old_node(len(stream) + oi, oi, L_b)

            H = K_MAX // 2
            if len(stream) == 2:
                # interleave: node-0 epilogue phase A rides between node-1's
                # matmul halves; only node-1's epilogue remains as tail.
                ps0 = emit_stream_mms(0, stream[0][1], 0, K_MAX)
                f0 = stream_fold(0, ps0, stream[0][1])
                w0 = epi_A(0, f0[:], K_MAX)
                ps1 = emit_stream_mms(1, stream[1][1], 0, K_MAX)
                epi_B(w0, f0[:], K_MAX)
                f1 = stream_fold(1, ps1, stream[1][1])
                tree_epilogue(1, f1[:], K_MAX)
            else:
                st_ps = [emit_stream_mms(i, L, 0, K_MAX)
                         for i, (s, L, _d) in enumerate(stream)]
                for i, (s, L, _d) in enumerate(stream):
                    f_sb = stream_fold(i, st_ps[i], L)
                    tree_epilogue(i, f_sb[:], K_MAX)

            # ================= final output =================
            acc_sb = spool.tile([HPC, F], f32, tag="acc_sb")
            nc.scalar.copy(acc_sb[:], out_ps[:])
            # head h's output lives at acc_sb[h, h*64:(h+1)*64]; DMA handles the
            # partition-base-1 read that compute engines can't.  Two rings so
            # the two descriptor generations overlap.
            nc.scalar.dma_start(o[0:1, :], acc_sb[0:1, 0:HEAD_DIM])
            nc.sync.dma_start(o[1:2, :], acc_sb[1:2, HEAD_DIM : 2 * HEAD_DIM])

    nc.compile()
    return nc


def _stream_host_layout(V8, L):
    """Rearrange one stream node's tokens to the SBUF layout.
    token t = (cb*128 + 2j + b)*64 + r lands at [p = r + 64b][j, ...]:
    DR: free (j, i, g, f) with cb = i*ng + g; plain: free (j, cb, f)."""
    nb = L // BLK
    A = V8.reshape(nb, K_MAX, 2, K_MAX, F)          # (cb, j, b, r, f)
    if _use_dr(L):
        ng = nb // 2
        A = A.reshape(2, ng, K_MAX, 2, K_MAX, F)    # (i, g, j, b, r, f)
        A = A.transpose(3, 4, 2, 0, 1, 5)           # (b, r, j, i, g, f)
    else:
        A = A.transpose(2, 3, 1, 0, 4)              # (b, r, j, cb, f)
    return np.ascontiguousarray(A.reshape(CHUNK, K_MAX * nb * F))


def _make_in_maps(v_tokens, q_new, depth_proj_w, depth_temp, pos):
    stream, old, small = _split_nodes(pos)
    tree = stream + old + small
    NT = len(tree)
    OFF, CB_W = _cblob_layout(NT)
    n_loc = min(pos, LOCAL_WINDOW)
    NLC = n_loc // CHUNK
    lstart = pos - n_loc

    _, np_dt = _mode_dt()
    pp = np.zeros((CHUNK, 2, K_MAX), np.float32)
    for p in range(CHUNK):
        pp[p, :, p % K_MAX] = 1.0
    pp8 = np.ascontiguousarray(pp.astype(np_dt).reshape(CHUNK, 2 * K_MAX))
    sel = np.tile(np.eye(K_MAX, dtype=np.float32), (CHUNK // K_MAX, 1))

    ms = [float(K_MAX) / L if L > K_MAX else 1.0 for (_s, L, _d) in tree]
    sp = np.log1p(np.exp(depth_temp.astype(np.float64)))
    rs_eff = np.array(
        [ms[n] / ((sp[d] + 1e-6) * math.sqrt(HEAD_DIM))
         for n, (_s, _L, d) in enumerate(tree)], np.float32,
    ) if NT else np.zeros((1,), np.float32)

    in_maps = []
    for c in range(N_CORES):
        v_c = np.ascontiguousarray(
            v_tokens[:, HPC * c : HPC * (c + 1), :]
        ).reshape(NTOK, F)
        q_c = q_new[0, HPC * c : HPC * (c + 1), :]          # (2, 64)

        cbl = np.zeros((CHUNK, CB_W), np.float32)
        cbl[:, OFF["ident"] : OFF["ident"] + CHUNK] = np.eye(CHUNK)
        for h in range(HPC):
            cbl[h * HEAD_DIM : (h + 1) * HEAD_DIM, OFF["qbd"] + h] = q_c[h]
        for n, (_s, _L, d) in enumerate(tree):
            qd = q_c + q_c @ depth_proj_w[d].T              # (2, 64)
            for h in range(HPC):
                cbl[h * HEAD_DIM : (h + 1) * HEAD_DIM,
                    OFF["qdT"] + n * HPC + h] = qd[h]
        cbl[0:HPC, OFF["rs"] : OFF["rs"] + max(NT, 1)] = rs_eff[None, :]

        im = {"pp8": pp8, "cblob": cbl}
        NS = len(small)
        vres = np.zeros((CHUNK, NLC + max(NS, 1), F), np.float32)
        vres[:, 0:NLC, :] = (
            v_c[lstart : lstart + n_loc].reshape(NLC, CHUNK, F).transpose(1, 0, 2)
        )
        for si, (s, L_s, _d) in enumerate(small):
            vres[0:L_s, NLC + si, :] = v_c[s : s + L_s]
        im["vres"] = np.ascontiguousarray(vres.reshape(CHUNK, -1))
        old_nch = [L_b // CHUNK for (_s, L_b, _d) in old]
        voldb = np.zeros((CHUNK, K_MAX + sum(old_nch) * F), np.float32)
        voldb[:, 0:K_MAX] = sel
        off = K_MAX
        for oi, (s, L_b, _d) in enumerate(old):
            nch = L_b // CHUNK
            voldb[:, off : off + nch * F] = (
                v_c[s : s + L_b].reshape(nch, CHUNK, F)
                .transpose(1, 0, 2).reshape(CHUNK, nch * F)
            )
            off += nch * F
        im["voldb"] = np.ascontiguousarray(voldb)
        for i, (s, L, _d) in enumerate(stream):
            im[f"a{i}"] = _stream_host_layout(v_c[s : s + L].astype(np_dt), L)
        in_maps.append(im)
    return in_maps


def kernel(v_tokens, q_new, depth_proj_w, depth_temp, n_tokens, _profile=False):
    global _last_results
    v_tokens = np.asarray(v_tokens, dtype=np.float32)
    q_new = np.asarray(q_new, dtype=np.float32)
    depth_proj_w = np.asarray(depth_proj_w, dtype=np.float32)
    depth_temp = np.asarray(depth_temp, dtype=np.float32)
    pos = int(n_tokens)

    nc = _build_program(pos)
    in_maps = _make_in_maps(v_tokens, q_new, depth_proj_w, depth_temp, pos)
    res = run_bass_kernel_spmd(
        nc, in_maps, core_ids=list(range(N_CORES)), trace=_profile
    )
    _last_results = res

    out = np.zeros((1, NUM_HEADS, HEAD_DIM), np.float32)
    for c in range(N_CORES):
        out[0, HPC * c : HPC * (c + 1), :] = res.results[c]["o"]
    return out


# revision 18
# speedup vs baseline: 1.4493x; 1.4493x over previous
"""Trainium2 Bass kernel for nn_DCWTv2InferenceCache (segment-tree cached attention).

Sharding: tensor-parallel over the 16-head axis -> 8 cores x 2 heads.
The two big segment-tree nodes (98% of the value-cache bytes) stream from
HBM as fp8-e4m3 in a host-pre-arranged layout: SBUF partition = (token%64)
+ 64*(chunk-row parity), so ONE fixed fold matrix PP[p,k]=(k==p%64) serves
every block-sum matmul (walrus collapses the repeated LDWEIGHTS to a no-op)
and DoubleRow contracts two c-blocks per instruction (4 tokens/cycle).
Local window / small nodes / mid nodes stay f32, host-pre-arranged to
partition-major contiguous layouts so their DMAs are descriptor-light.
All input-derived scalars (depth-projected queries, softmax scales with
block-mean and 1/NT folded in) are host-computed and shipped in one consts
blob.  Output is head-sharded (2, 64) per core, gathered on host.  No
cross-device communication.
"""

import math
import os
import sys

if "/opt/trn_rl_repo" not in sys.path:
    sys.path.insert(0, "/opt/trn_rl_repo")

import numpy as np
import ml_dtypes

import concourse.bass as bass
import concourse.mybir as mybir
import concourse.tile as tile
from concourse import bacc
from concourse.bass_utils import run_bass_kernel_spmd

# --- problem constants (from the reference nn.Module) ---
MAX_LEN = 65536
NUM_HEADS = 16
HEAD_DIM = 64
K_MAX = 64
LOCAL_WINDOW = 512
LOG_N = 17
LEAF_START = 2**LOG_N

N_CORES = 8
HPC = NUM_HEADS // N_CORES        # heads per core = 2
F = HPC * HEAD_DIM                # feature width per core = 128
NTOK = 50000                      # v_tokens buffer length

CHUNK = 128                       # tokens per matmul tile (partition dim)
BLK = CHUNK * K_MAX               # 8192 tokens per c-block (stream path)

# stream-path arithmetic mode: "fp8dr" | "fp8" | "bf16"
MODE = os.environ.get("DCWT_MODE", "fp8dr")

f32 = mybir.dt.float32
AF = mybir.ActivationFunctionType
AX = mybir.AxisListType

_last_results = None  # stash for test harness introspection


def cover_set(pos):
    """O(log n) segment-tree nodes covering prefix [0..pos-1]: (start, L, depth),
    ascending start / descending L (binary decomposition of pos)."""
    if pos <= 0:
        return []
    l, r = LEAF_START, LEAF_START + min(pos, MAX_LEN)
    out = []
    while l < r:
        if l & 1:
            d = LOG_N - int(math.floor(math.log2(l)))
            out.append(((l << d) - LEAF_START, 1 << d, d))
            l += 1
        if r & 1:
            r -= 1
            d = LOG_N - int(math.floor(math.log2(r)))
            out.append(((r << d) - LEAF_START, 1 << d, d))
        l >>= 1
        r >>= 1
    return sorted(out)


def _split_nodes(pos):
    """stream nodes (L >= BLK, fp8 path) / old nodes (K_MAX < L < BLK, f32r
    PE path) / small nodes (L <= K_MAX, raw)."""
    nodes = cover_set(pos)
    stream = [(s, L, d) for (s, L, d) in nodes if L >= BLK]
    old = [(s, L, d) for (s, L, d) in nodes if K_MAX < L < BLK]
    small = [(s, L, d) for (s, L, d) in nodes if L <= K_MAX]
    return stream, old, small


def _cblob_layout(NT):
    """Column offsets inside the packed (128, W) f32 constants blob."""
    nt = max(NT, 1)
    off = {"ident": 0, "qbd": 128, "qdT": 130, "rs": 130 + 2 * nt}
    return off, 130 + 3 * nt


def _mode_dt():
    if MODE == "bf16":
        return mybir.dt.bfloat16, np.dtype(ml_dtypes.bfloat16)
    return mybir.dt.float8e4, np.dtype(ml_dtypes.float8_e4m3)


def _use_dr(L):
    return MODE == "fp8dr" and (L // BLK) >= 2


def _build_program(pos):
    """Build the single-core Bass/Tile program (same program for all 8 cores)."""
    stream, old, small = _split_nodes(pos)
    tree = stream + old + small
    NT = len(tree)
    n_loc = min(pos, LOCAL_WINDOW)
    assert n_loc % CHUNK == 0, "local window must be chunk-aligned for this build"
    NLC = n_loc // CHUNK
    inv_sqrt_d = 1.0 / math.sqrt(HEAD_DIM)
    st_dt, _ = _mode_dt()

    nc = bacc.Bacc("TRN2", target_bir_lowering=False, debug=False)

    # --- HBM tensors ---
    a_d = [
        nc.dram_tensor(f"a{i}", [CHUNK, (L // CHUNK) * F], st_dt,
                       kind="ExternalInput")
        for i, (s, L, _d) in enumerate(stream)
    ]
    pp8_d = nc.dram_tensor("pp8", [CHUNK, 2 * K_MAX], st_dt, kind="ExternalInput")
    OFF, CB_W = _cblob_layout(NT)
    cblob_d = nc.dram_tensor("cblob", [CHUNK, CB_W], f32, kind="ExternalInput")
    # f32 residual blob: local window chunks + (row-truncated) small nodes
    NS = len(small)
    vres_d = nc.dram_tensor("vres", [CHUNK, (NLC + max(NS, 1)) * F], f32,
                            kind="ExternalInput")
    # f32r blob: old-path fold matrix + old-node chunks, one DMA
    old_nch = [L_b // CHUNK for (_s, L_b, _d) in old]
    VO_W = K_MAX + sum(old_nch) * F
    vold_d = nc.dram_tensor("voldb", [CHUNK, VO_W], mybir.dt.float32r,
                            kind="ExternalInput")
    o = nc.dram_tensor("o", [HPC, HEAD_DIM], f32, kind="ExternalOutput")

    # host-baked python-float constants per tree node
    ms = [float(K_MAX) / L if L > K_MAX else 1.0 for (_s, L, _d) in tree]
    zfac = [float(NT) / m for m in ms]

    with tile.TileContext(nc) as tc:
        with (
            tc.tile_pool(name="consts", bufs=1) as cpool,
            tc.tile_pool(name="ep_sb", bufs=2) as spool,
            tc.tile_pool(name="xsb", bufs=3) as xpool,
            tc.tile_pool(name="acc_ps", bufs=1, space=bass.MemorySpace.PSUM) as apool,
            tc.tile_pool(name="ep_ps", bufs=1, space=bass.MemorySpace.PSUM) as eppool,
            tc.tile_pool(name="out_ps", bufs=1, space=bass.MemorySpace.PSUM) as opool,
        ):
            # ---- DMA issue: everything rides the sync HWDGE ring, which is
            # FIFO and strictly outprioritizes the scalar ring at the SDMA
            # level (a big stream on one ring STARVES the other).  Order:
            # consts + residuals first (~0.85 MB), then the fp8 stream in
            # j-halves so the PE chases it at fine grain.
            pp8t = cpool.tile([CHUNK, 2, K_MAX], st_dt)
            nc.sync.dma_start(pp8t[:], pp8_d[:])
            cb = cpool.tile([CHUNK, CB_W], f32)
            nc.sync.dma_start(cb[:], cblob_d[:])
            ident_sb = cb[:, OFF["ident"] : OFF["ident"] + CHUNK]
            qbd_sb = cb[:, OFF["qbd"] : OFF["qbd"] + HPC]

            def qdT_slice(n):
                return cb[:, OFF["qdT"] + n * HPC : OFF["qdT"] + (n + 1) * HPC]

            rs_sb = cb[0:HPC, OFF["rs"] : OFF["rs"] + max(NT, 1)]

            vres = cpool.tile([CHUNK, NLC + max(NS, 1), F], f32)
            nc.sync.dma_start(vres[:], vres_d[:])
            voldb = cpool.tile([CHUNK, VO_W], mybir.dt.float32r)
            nc.sync.dma_start(voldb[:], vold_d[:])

            a_tiles = []
            for i, (s, L, _d) in enumerate(stream):
                nb = L // BLK
                if _use_dr(L):
                    at = cpool.tile([CHUNK, K_MAX, 2, (nb // 2) * F], st_dt,
                                    name=f"a{i}t", tag=f"a{i}t")
                    h1, h2 = at[:, 0 : K_MAX // 2, :, :], at[:, K_MAX // 2 :, :, :]
                else:
                    at = cpool.tile([CHUNK, K_MAX, nb * F], st_dt,
                                    name=f"a{i}t", tag=f"a{i}t")
                    h1, h2 = at[:, 0 : K_MAX // 2, :], at[:, K_MAX // 2 :, :]
                half = (K_MAX // 2) * nb * F
                nc.sync.dma_start(h1, a_d[i][:, 0:half])
                nc.sync.dma_start(h2, a_d[i][:, half:])
                a_tiles.append(at)
            old_off = []
            off = K_MAX
            for nch in old_nch:
                old_off.append(off)
                off += nch * F

            # ---- cross-node output accumulator (2, 128) PSUM ----
            out_ps = opool.tile([HPC, F], f32)
            n_out_mm = NT + NLC
            out_mm = [0]  # running count, for start/stop flags

            def out_matmul(wT_sb_ap, f_sb_ap):
                nc.tensor.matmul(
                    out_ps[:], wT_sb_ap, f_sb_ap,
                    start=(out_mm[0] == 0), stop=(out_mm[0] == n_out_mm - 1),
                )
                out_mm[0] += 1

            def softmax_weights(s_ps_ap, K, node_i):
                """exp(scale*s) -> normalize; tree weights fold 1/NT and the
                block-mean factor via zfac (host-baked).  No max-subtraction:
                logits here are provably small."""
                ebd = xpool.tile([HPC, K], f32, tag="esb")
                zt = xpool.tile([HPC, 1], f32, tag="zt")
                if node_i >= 0:
                    nc.scalar.activation(
                        ebd[:], s_ps_ap, AF.Exp,
                        scale=rs_sb[:, node_i : node_i + 1], accum_out=zt[:],
                    )
                    zs = xpool.tile([HPC, 1], f32, tag="zs")
                    nc.scalar.mul(zs[:], zt[:], zfac[node_i])
                    zt = zs
                else:
                    nc.scalar.activation(
                        ebd[:], s_ps_ap, AF.Exp, scale=inv_sqrt_d,
                        accum_out=zt[:],
                    )
                rz = xpool.tile([HPC, 1], f32, tag="rz")
                nc.vector.reciprocal(rz[:], zt[:])
                w_sb = xpool.tile([HPC, K], f32, tag="wsb")
                nc.vector.tensor_scalar_mul(w_sb[:], ebd[:], rz[:])
                return w_sb

            def epi_A(node_i, f_sb_ap, K):
                """Transpose f, logits, softmax -> weights (PE ops: T1, M1)."""
                fT_ps = eppool.tile([F, K_MAX], f32, tag="fT_ps", bufs=1)
                nc.tensor.transpose(fT_ps[:, 0:K], f_sb_ap, ident_sb[0:K, 0:K])
                fT_sb = spool.tile([F, K_MAX], f32, tag="fT_sb")
                nc.scalar.copy(fT_sb[:, 0:K], fT_ps[:, 0:K])
                s_ps = eppool.tile([HPC, K_MAX], f32, tag="s_ps", bufs=1)
                nc.tensor.matmul(
                    s_ps[:, 0:K], qdT_slice(node_i), fT_sb[:, 0:K],
                    start=True, stop=True,
                )
                return softmax_weights(s_ps[:, 0:K], K, node_i)

            def epi_B(w_sb, f_sb_ap, K):
                """Weights transpose + output accumulation (PE ops: T2, M2)."""
                wT_ps = eppool.tile([K_MAX, HPC], f32, tag="wT_ps")
                nc.tensor.transpose(wT_ps[0:K, :], w_sb[:], ident_sb[0:HPC, 0:HPC])
                wT_sb = spool.tile([K_MAX, HPC], f32, tag="wT_sb")
                nc.scalar.copy(wT_sb[0:K, :], wT_ps[0:K, :])
                out_matmul(wT_sb[0:K, :], f_sb_ap)

            def tree_epilogue(node_i, f_sb_ap, K):
                epi_B(epi_A(node_i, f_sb_ap, K), f_sb_ap, K)

            # ================= emission schedule =================
            def emit_local():
                fTl_ps = eppool.tile([F, NLC * CHUNK], f32, tag="fT_ps", bufs=1)
                for c in range(NLC):
                    nc.tensor.transpose(
                        fTl_ps[:, c * CHUNK : (c + 1) * CHUNK], vres[:, c, :],
                        ident_sb[:],
                    )
                fTl_sb = spool.tile([F, NLC * CHUNK], f32, tag="fTl_sb")
                nc.scalar.copy(fTl_sb[:], fTl_ps[:])
                sl_ps = eppool.tile([HPC, NLC * CHUNK], f32, tag="s_ps", bufs=1)
                nc.tensor.matmul(sl_ps[:], qbd_sb, fTl_sb[:], start=True, stop=True)
                wl_sb = softmax_weights(sl_ps[:], n_loc, -1)
                for c in range(NLC):
                    wTl_ps = eppool.tile([CHUNK, HPC], f32, tag="wT_ps")
                    nc.tensor.transpose(
                        wTl_ps[:], wl_sb[:, c * CHUNK : (c + 1) * CHUNK],
                        ident_sb[0:HPC, 0:HPC],
                    )
                    wTl_sb = spool.tile([CHUNK, HPC], f32, tag="wTl_sb")
                    nc.scalar.copy(wTl_sb[:], wTl_ps[:])
                    out_matmul(wTl_sb[:], vres[:, c, :])

            def emit_old_node(node_i, oi, L):
                nch = L // CHUNK
                base = old_off[oi]
                ps2 = apool.tile([K_MAX, 2, F], f32, tag="acc")
                done, c = 0, 0
                while c < nch:
                    w = 2 if c + 2 <= nch else 1
                    nc.tensor.matmul(
                        ps2[:, 0:w, :], voldb[:, 0:K_MAX],
                        voldb[:, base + c * F : base + (c + w) * F],
                        start=(done == 0), stop=(done + w == nch),
                    )
                    done += w
                    c += w
                f_sb = spool.tile([K_MAX, F], f32, tag="fold")
                if nch > 1:
                    g = spool.tile([K_MAX, 2, F], f32, tag="gfold")
                    nc.scalar.copy(g[:], ps2[:])
                    nc.vector.tensor_add(f_sb[:], g[:, 0, :], g[:, 1, :])
                else:
                    nc.scalar.copy(f_sb[:], ps2[:, 0, :])
                tree_epilogue(node_i, f_sb[:], K_MAX)

            def emit_stream_mms(i, L, jlo, jhi):
                """Fold matmuls for stream node i, j in [jlo, jhi).  All MMs
                share the fixed PP weights -> LDWEIGHTS collapses to a no-op."""
                nb = L // BLK
                at = a_tiles[i]
                ncol = nb // 2 if _use_dr(L) else nb
                ps = apool.tile([K_MAX, ncol * F], f32, tag=f"st{i}")
                if _use_dr(L):
                    dr = mybir.MatmulPerfMode.DoubleRow
                    for j in range(jlo, jhi):
                        nc.tensor.matmul(
                            ps[:], pp8t[:], at[:, j, :, :],
                            start=(j == 0), stop=(j == K_MAX - 1), perf_mode=dr,
                        )
                else:
                    for j in range(jlo, jhi):
                        nc.tensor.matmul(
                            ps[:], pp8t[:, 0, :], at[:, j, :],
                            start=(j == 0), stop=(j == K_MAX - 1),
                        )
                return ps

            def stream_fold(i, ps, L):
                """Combine c-group partial sums -> (64, F) raw block-sum."""
                ncol = (L // BLK) // 2 if _use_dr(L) else L // BLK
                f_sb = spool.tile([K_MAX, F], f32, tag="fold")
                if ncol == 1:
                    nc.scalar.copy(f_sb[:], ps[:])
                    return f_sb
                g = spool.tile([K_MAX, ncol * F], f32, tag="gfold")
                nc.scalar.copy(g[:], ps[:])
                if ncol == 2:
                    nc.vector.tensor_add(f_sb[:], g[:, 0:F], g[:, F : 2 * F])
                else:  # ncol == 4
                    ha = spool.tile([K_MAX, F], f32, tag="ha")
                    nc.vector.tensor_add(ha[:], g[:, 0:F], g[:, F : 2 * F])
                    hb = spool.tile([K_MAX, F], f32, tag="hb")
                    nc.vector.tensor_add(hb[:], g[:, 2 * F : 3 * F],
                                         g[:, 3 * F : 4 * F])
                    nc.vector.tensor_add(f_sb[:], ha[:], hb[:])
                return f_sb

            emit_local()
            for si, (start_s, L_s, _d) in enumerate(small):
                tree_epilogue(len(stream) + len(old) + si,
                              vres[0:L_s, NLC + si, :], L_s)
            for oi, (start_b, L_b, _d) in enumerate(old):
                emit_old_node(len(stream) + oi, oi, L_b)

            H = K_MAX // 2
            if len(stream) == 2:
                # interleave: node-0 epilogue phase A rides between node-1's
                # matmul halves; only node-1's epilogue remains as tail.
                ps0 = emit_stream_mms(0, stream[0][1], 0, K_MAX)
                f0 = stream_fold(0, ps0, stream[0][1])
                w0 = epi_A(0, f0[:], K_MAX)
                ps1 = emit_stream_mms(1, stream[1][1], 0, K_MAX)
                epi_B(w0, f0[:], K_MAX)
                f1 = stream_fold(1, ps1, stream[1][1])
                tree_epilogue(1, f1[:], K_MAX)
            else:
                st_ps = [emit_stream_mms(i, L, 0, K_MAX)
                         for i, (s, L, _d) in enumerate(stream)]
                for i, (s, L, _d) in enumerate(stream):
                    f_sb = stream_fold(i, st_ps[i], L)
                    tree_epilogue(i, f_sb[:], K_MAX)

            # ================= final output =================
            acc_sb = spool.tile([HPC, F], f32, tag="acc_sb")
            nc.scalar.copy(acc_sb[:], out_ps[:])
            # head h's output lives at acc_sb[h, h*64:(h+1)*64]; DMA handles the
            # partition-base-1 read that compute engines can't.  Two rings so
            # the two descriptor generations overlap.
            nc.scalar.dma_start(o[0:1, :], acc_sb[0:1, 0:HEAD_DIM])
            nc.sync.dma_start(o[1:2, :], acc_sb[1:2, HEAD_DIM : 2 * HEAD_DIM])

    nc.compile()
    return nc


def _stream_host_layout(V8, L):
    """Rearrange one stream node's tokens to the SBUF layout.
    token t = (cb*128 + 2j + b)*64 + r lands at [p = r + 64b][j, ...]:
    DR: free (j, i, g, f) with cb = i*ng + g; plain: free (j, cb, f)."""
    nb = L // BLK
    A = V8.reshape(nb, K_MAX, 2, K_MAX, F)          # (cb, j, b, r, f)
    if _use_dr(L):
        ng = nb // 2
        A = A.reshape(2, ng, K_MAX, 2, K_MAX, F)    # (i, g, j, b, r, f)
        A = A.transpose(3, 4, 2, 0, 1, 5)           # (b, r, j, i, g, f)
    else:
        A = A.transpose(2, 3, 1, 0, 4)              # (b, r, j, cb, f)
    return np.ascontiguousarray(A.reshape(CHUNK, K_MAX * nb * F))


def _make_in_maps(v_tokens, q_new, depth_proj_w, depth_temp, pos):
    stream, old, small = _split_nodes(pos)
    tree = stream + old + small
    NT = len(tree)
    OFF, CB_W = _cblob_layout(NT)
    n_loc = min(pos, LOCAL_WINDOW)
    NLC = n_loc // CHUNK
    lstart = pos - n_loc

    _, np_dt = _mode_dt()
    pp = np.zeros((CHUNK, 2, K_MAX), np.float32)
    for p in range(CHUNK):
        pp[p, :, p % K_MAX] = 1.0
    pp8 = np.ascontiguousarray(pp.astype(np_dt).reshape(CHUNK, 2 * K_MAX))
    sel = np.tile(np.eye(K_MAX, dtype=np.float32), (CHUNK // K_MAX, 1))

    ms = [float(K_MAX) / L if L > K_MAX else 1.0 for (_s, L, _d) in tree]
    sp = np.log1p(np.exp(depth_temp.astype(np.float64)))
    rs_eff = np.array(
        [ms[n] / ((sp[d] + 1e-6) * math.sqrt(HEAD_DIM))
         for n, (_s, _L, d) in enumerate(tree)], np.float32,
    ) if NT else np.zeros((1,), np.float32)

    in_maps = []
    for c in range(N_CORES):
        v_c = np.ascontiguousarray(
            v_tokens[:, HPC * c : HPC * (c + 1), :]
        ).reshape(NTOK, F)
        q_c = q_new[0, HPC * c : HPC * (c + 1), :]          # (2, 64)

        cbl = np.zeros((CHUNK, CB_W), np.float32)
        cbl[:, OFF["ident"] : OFF["ident"] + CHUNK] = np.eye(CHUNK)
        for h in range(HPC):
            cbl[h * HEAD_DIM : (h + 1) * HEAD_DIM, OFF["qbd"] + h] = q_c[h]
        for n, (_s, _L, d) in enumerate(tree):
            qd = q_c + q_c @ depth_proj_w[d].T              # (2, 64)
            for h in range(HPC):
                cbl[h * HEAD_DIM : (h + 1) * HEAD_DIM,
                    OFF["qdT"] + n * HPC + h] = qd[h]
        cbl[0:HPC, OFF["rs"] : OFF["rs"] + max(NT, 1)] = rs_eff[None, :]

        im = {"pp8": pp8, "cblob": cbl}
        NS = len(small)
        vres = np.zeros((CHUNK, NLC + max(NS, 1), F), np.float32)
        vres[:, 0:NLC, :] = (
            v_c[lstart : lstart + n_loc].reshape(NLC, CHUNK, F).transpose(1, 0, 2)
        )
        for si, (s, L_s, _d) in enumerate(small):
            vres[0:L_s, NLC + si, :] = v_c[s : s + L_s]
        im["vres"] = np.ascontiguousarray(vres.reshape(CHUNK, -1))
        old_nch = [L_b // CHUNK for (_s, L_b, _d) in old]
        voldb = np.zeros((CHUNK, K_MAX + sum(old_nch) * F), np.float32)
        voldb[:, 0:K_MAX] = sel
        off = K_MAX
        for oi, (s, L_b, _d) in enumerate(old):
            nch = L_b // CHUNK
            voldb[:, off : off + nch * F] = (
                v_c[s : s + L_b].reshape(nch, CHUNK, F)
                .transpose(1, 0, 2).reshape(CHUNK, nch * F)
            )
            off += nch * F
        im["voldb"] = np.ascontiguousarray(voldb)
        for i, (s, L, _d) in enumerate(stream):
            im[f"a{i}"] = _stream_host_layout(v_c[s : s + L].astype(np_dt), L)
        in_maps.append(im)
    return in_maps


def kernel(v_tokens, q_new, depth_proj_w, depth_temp, n_tokens, _profile=False):
    global _last_results
    v_tokens = np.asarray(v_tokens, dtype=np.float32)
    q_new = np.asarray(q_new, dtype=np.float32)
    depth_proj_w = np.asarray(depth_proj_w, dtype=np.float32)
    depth_temp = np.asarray(depth_temp, dtype=np.float32)
    pos = int(n_tokens)

    nc = _build_program(pos)
    in_maps = _make_in_maps(v_tokens, q_new, depth_proj_w, depth_temp, pos)
    res = run_bass_kernel_spmd(
        nc, in_maps, core_ids=list(range(N_CORES)), trace=_profile
    )
    _last_results = res

    out = np.zeros((1, NUM_HEADS, HEAD_DIM), np.float32)
    for c in range(N_CORES):
        out[0, HPC * c : HPC * (c + 1), :] = res.results[c]["o"]
    return out
